# revision 15
# baseline (speedup 1.0000x reference)
"""Trainium2 Bass kernel for nn_Conv2DSum (logconv1x1_2d / SPN sum layer).

Math: out[b,h,w,s] = logsumexp_c( x[b,h,w,c] + log_softmax(acc)[c,s] )
Since w = softmax(acc) along c sums to 1, the result equals
    out = log( exp(x) @ w )
which is a convex combination of exp(x_c) — numerically safe in fp32/fp16
range for N(0,1)-scale inputs (no max-subtraction needed).

V3 strategy (per core, batch-sharded 8 ways: 4 batches = 65536 rows x 32 ch),
memory-regime: all HBM I/O in fp16 (half the bytes of the fp32 baseline),
PE runs fp16 (1 cycle/row vs 4 for fp32), and the exp() is computed on the
otherwise-idle DVE via the classic float bit-trick so ACT only runs the
exact Ln:

  - x fp16 tiles [128, 2048]; PE transpose of [128,128] slices puts
    (4 rows x 32 ch) on partitions; fp16 transpose writes fp16 PSUM, so a
    whole tile's transpose fits 2 PSUM banks.
  - exp via DVE: i16 = round(x * 1024/ln2 + (15-sigma)*1024) written as
    int16; those bits reinterpreted as fp16 are ~exp(x) (max ~4% relative,
    deterministic, mostly averaged out by the 32-channel weighted sum).
    One tensor_scalar per tile, PSUM fp16 -> SBUF int16.
  - one matmul per [128,128] slice: stationary = p~ (bitcast fp16),
    moving = 128x128 block-diagonal weight (4 copies of the 32x32 softmax
    matrix) so 4 row-groups go in a single K=128,M=128,N=128 fp16 matmul.
  - exact Ln via ScalarE ACT (PSUM fp32 -> SBUF fp16), [128,1024] per
    2-bank PSUM group.
  - out fp16 -> HBM; host widens to fp32.

End-to-end rel err (host-simulated + device-verified) ~8.7e-3 vs the 2e-2
gate, dominated by the exp bit-trick; exact-exp fallback (USE_TRICK_EXP =
False) runs exp on ACT instead (~1.5e-3 but ~9us slower).
"""

from contextlib import ExitStack

import numpy as np

import concourse.bass as bass
import concourse.tile as tile
from concourse import mybir

# Problem shape (hardcoded per contest rules)
B, H, W, C_IN, N_SUMS = 32, 128, 128, 32, 32
N_CORES = 8
B_PER_CORE = B // N_CORES              # 4
ROWS_PER_CORE = B_PER_CORE * H * W     # 65536
FREE = 2048                            # big-tile free dim (64 rows x 32 ch)
N_TILES = ROWS_PER_CORE * C_IN // (128 * FREE)   # 8
N_SLICES = FREE // 128                 # 16 slices of [128,128] per big tile
SLICES_PER_BANK = 8                    # 8 slices per [128,1024] fp32 PSUM group
N_BANKGROUPS = N_SLICES // SLICES_PER_BANK       # 2

F32 = mybir.dt.float32
F16 = mybir.dt.float16
I16 = mybir.dt.int16

USE_TRICK_EXP = True

# exp(x) ~= bitcast_fp16(int16(round(A16*x + B16)))
_SIGMA = 0.0455
A16 = 1024.0 / float(np.log(2.0))
B16 = (15.0 - _SIGMA) * 1024.0


# Per-core HBM layout is partition-major: [128, 16384] fp16 where partition
# p owns rows [p*512, (p+1)*512) of the core's 65536 rows (each row = 32 ch).
# This gives 32KB contiguous HBM per partition, so a handful of big DMAs
# (4-8KB contiguous per partition each) replace 32 small ones: DMA_DIRECT2D
# costs ~600ns of Sync-engine time apiece, and 2KB-run descriptors kept the
# DMA engines at only ~57% busy in the V3 trace.
TOTAL_FREE = N_TILES * FREE            # 16384
# (col_offset, width) of each input DMA; first two small so compute starts
# after ~0.25MB instead of 1MB. All chunks stay resident in SBUF (32KB per
# partition total), so every input DMA issues with no WAR wait and the
# in-order Sync queue never head-of-line blocks an input load behind an
# output store.
X_CHUNKS = [(0, 1024), (1024, 1024), (2048, 2048), (4096, 4096), (8192, 4096), (12288, 4096)]
# compute tiles (col, width): small first tiles shorten the lead-in (first
# transpose burst only waits 0.25MB of DMA), small last tiles shorten the
# serial drain tail (T->DVE->MM->Ln->DMA on a 1024-wide tile is ~half the
# latency of a 2048 one).
C_TILES = [(0, 1024), (1024, 1024)] + [
    (c, 2048) for c in range(2048, 14336, 2048)
] + [(14336, 1024), (15360, 1024)]


def build_kernel(nc: bass.Bass, repeat: int = 1):
    x_d = nc.dram_tensor("x", [128, TOTAL_FREE], F16, kind="ExternalInput").ap()
    wblk_d = nc.dram_tensor("w_blk", [128, 128], F16, kind="ExternalInput").ap()
    ident_d = nc.dram_tensor("ident", [128, 128], F16, kind="ExternalInput").ap()
    out_d = nc.dram_tensor("out", [128, TOTAL_FREE], F16, kind="ExternalOutput").ap()

    with tile.TileContext(nc) as tc, ExitStack() as ctx:
        const_pool = ctx.enter_context(tc.tile_pool(name="const", bufs=1))
        # SBUF is ~64KB/partition usable here; pool slots are sized to the
        # largest tile, so segregate x chunks by width to stay within budget.
        x_pools = {}
        for width in sorted({w for _, w in X_CHUNKS}):
            n = sum(1 for _, w in X_CHUNKS if w == width)
            x_pools[width] = ctx.enter_context(
                tc.tile_pool(name=f"x{width}", bufs=n)
            )
        p_pool = ctx.enter_context(tc.tile_pool(name="p", bufs=2))
        o_pool = ctx.enter_context(tc.tile_pool(name="o", bufs=3))
        # psT: whole-tile fp16 transpose = 4KB = 2 banks; psO: [128,1024] fp32
        # = 2 banks. 2 bufs each -> all 8 banks.
        psT_pool = ctx.enter_context(tc.tile_pool(name="psT", bufs=2, space="PSUM"))
        psO_pool = ctx.enter_context(tc.tile_pool(name="psO", bufs=2, space="PSUM"))

        def load_chunk(c):
            off, width = X_CHUNKS[c]
            xt = x_pools[width].tile([128, width], F16, tag=f"x{c}")
            nc.sync.dma_start(xt[:], x_d[:, off : off + width])
            return xt

        # ident gates the first transpose and is tiny: load it first, then
        # x chunk 0, then wblk (not needed until the first weight MM), then
        # the remaining chunks — all unblocked, so the DMA engines saturate
        # on input immediately.
        ident = const_pool.tile([128, 128], F16, tag="ident")
        nc.sync.dma_start(ident[:], ident_d)
        xc0 = load_chunk(0)
        wblk = const_pool.tile([128, 128], F16, tag="wblk")
        nc.sync.dma_start(wblk[:], wblk_d)

        # walrus only allows ONE embedded sync-wait on a Matmult (the wait
        # rides the LDW struct), but Tile routinely needs 2+ on the first
        # matmul of a burst. A dummy bf16 ldweights is a PE *engine*
        # instruction (so its waits advance the PE engine's vector clock,
        # unlike a sequencer nop) with no PSUM side effects: we front-load
        # each matmul burst's cross-engine deps onto dummy LDWs, one dep
        # each, leaving at most the PSUM-bank WAW wait on the matmul itself.
        dummy_w = const_pool.tile([128, 8], mybir.dt.bfloat16, tag="dummyw")
        nc.gpsimd.memset(dummy_w[:], 1.0)

        # tiny dummy activation up front: forces the ~2.7us ACT table load
        # to overlap the first x DMA instead of sitting on the critical path
        warm_pool = ctx.enter_context(tc.tile_pool(name="warm", bufs=1))
        warm = warm_pool.tile([128, 1], F32, tag="warm")
        nc.scalar.activation(
            warm[:], dummy_w[:, 0:1], mybir.ActivationFunctionType.Ln
        )
        if not USE_TRICK_EXP:
            nc.scalar.activation(
                warm[:], dummy_w[:, 0:1], mybir.ActivationFunctionType.Exp
            )

        def chunk_of(col):
            for ci, (off, width) in enumerate(X_CHUNKS):
                if off <= col < off + width:
                    return ci, col - off
            raise AssertionError(col)

        for _rep in range(repeat):
            chunk_bufs = {0: xc0 if _rep == 0 else load_chunk(0)}
            for ci in range(1, len(X_CHUNKS)):
                chunk_bufs[ci] = load_chunk(ci)

            n_ct = len(C_TILES)

            def emit_transposes(t):
                col, width = C_TILES[t]
                psT = psT_pool.tile([128, width], F16)
                for k in range(width // 128):
                    ci, coff = chunk_of(col + k * 128)
                    nc.tensor.matmul(
                        psT[:, bass.ts(k, 128)],
                        chunk_bufs[ci][:, coff : coff + 128],
                        ident[:],
                        is_transpose=True,
                        start=(k % 4 == 0),
                        stop=(k % 4 == 3),
                    )
                return psT

            # software-pipelined PE stream: transposes run one tile ahead of
            # the weight matmuls so the in-order PE queue never stalls on the
            # DVE exp of the current tile.
            psTs = {0: emit_transposes(0), 1: emit_transposes(1)}
            for t in range(n_ct):
                col, width = C_TILES[t]
                psT = psTs.pop(t)
                if USE_TRICK_EXP:
                    pt = p_pool.tile([128, width], I16)
                    nc.vector.tensor_scalar(
                        pt[:],
                        psT[:],
                        A16,
                        B16,
                        op0=mybir.AluOpType.mult,
                        op1=mybir.AluOpType.add,
                    )
                    ptv = pt[:].bitcast(F16)
                else:
                    pt = p_pool.tile([128, width], F16)
                    nc.scalar.activation(
                        pt[:], psT[:], mybir.ActivationFunctionType.Exp
                    )
                    ptv = pt[:]
                if t + 2 < n_ct:
                    psTs[t + 2] = emit_transposes(t + 2)
                ot = o_pool.tile([128, width], F16)
                for b in range(width // (128 * SLICES_PER_BANK)):
                    psO = psO_pool.tile([128, 128 * SLICES_PER_BANK], F32)
                    for k in range(SLICES_PER_BANK):
                        j = b * SLICES_PER_BANK + k
                        nc.tensor.matmul(
                            psO[:, bass.ts(k, 128)],
                            ptv[:, bass.ts(j, 128)],
                            wblk[:],
                            start=(k % 4 == 0),
                            stop=(k % 4 == 3),
                        )
                    nc.scalar.activation(
                        ot[:, bass.ts(b, 128 * SLICES_PER_BANK)],
                        psO[:],
                        mybir.ActivationFunctionType.Ln,
                    )
                nc.sync.dma_start(out_d[:, col : col + width], ot[:])
    return nc


# walrus rejects >1 embedded sync-wait on engine-instruction structs
# (Matmult/Activation/DMA...). The NX sequencer executes embedded waits in
# stream order anyway, so spilling all-but-one wait onto dedicated nops
# immediately before the instruction is semantically identical.
_SPLIT_TYPES = (
    "InstMatmult",
    "InstLdweights",
    "InstActivation",
    "InstDMACopy",
    "InstMemset",
    "InstTensorTensor",
    "InstTensorScalarPtr",
    "InstCopy",
    "InstTensorReduce",
    "InstDrain",
    "InstNoOp",
)


def _split_embedded_waits(nc: bass.Bass):
    for fn in nc.m.functions:
        for blk in fn.blocks:
            insts = blk.instructions
            out = []
            for inst in insts:
                si = inst.sync_info
                if (
                    si is not None
                    and si.on_wait
                    and len(si.on_wait) > 1
                    and type(inst).__name__ in _SPLIT_TYPES
                ):
                    waits = list(si.on_wait)
                    for i, w in enumerate(waits[:-1]):
                        nop = mybir.InstNoOp(
                            name=f"{inst.name}-sw{i}",
                            engine=inst.engine,
                            sync_info=mybir.SyncInfo(on_wait=[w], on_update=[]),
                            bass_nofuse=True,
                        )
                        out.append(nop)
                    inst.sync_info = mybir.SyncInfo(
                        on_wait=[waits[-1]], on_update=list(si.on_update)
                    )
                out.append(inst)
            if len(out) != len(insts):
                blk.instructions[:] = out


def _host_weights(accumulators: np.ndarray) -> np.ndarray:
    """log_softmax over c of [1,1,Cin,S] accumulators -> exp -> block-diag."""
    acc = np.asarray(accumulators, dtype=np.float64)[0, 0]      # [Cin, S]
    m = acc.max(axis=0, keepdims=True)
    e = np.exp(acc - m)
    w = (e / e.sum(axis=0, keepdims=True)).astype(np.float16)   # [Cin, S]
    w_blk = np.zeros((128, 128), dtype=np.float16)
    for g in range(4):
        w_blk[32 * g : 32 * g + 32, 32 * g : 32 * g + 32] = w
    return w_blk


def make_in_maps(x: np.ndarray, acc: np.ndarray) -> list[dict]:
    x16 = np.ascontiguousarray(np.asarray(x).astype(np.float16))
    w_blk = _host_weights(np.asarray(acc, dtype=np.float32))
    ident = np.eye(128, dtype=np.float16)
    in_maps = []
    for c in range(N_CORES):
        xs = x16[c * B_PER_CORE : (c + 1) * B_PER_CORE]     # [4,128,128,32]
        # partition-major: partition p owns rows [p*512, (p+1)*512)
        xs = xs.reshape(128, TOTAL_FREE)
        in_maps.append({"x": xs, "w_blk": w_blk, "ident": ident})
    return in_maps


_CACHE: dict = {}


def make_bass():
    return bass.Bass("TRN2", debug=False, num_swdge_queues=4)


def get_nc():
    if "nc" not in _CACHE:
        nc = build_kernel(make_bass())
        # HW path only: CoreSim can't digest post-hoc inserted nops
        _split_embedded_waits(nc)
        _CACHE["nc"] = nc
    return _CACHE["nc"]


def kernel(**inputs: np.ndarray) -> np.ndarray:
    from concourse.bass_utils import run_bass_kernel_spmd

    in_maps = make_in_maps(inputs["x"], inputs["accumulators"])
    nc = get_nc()
    res = run_bass_kernel_spmd(nc, in_maps, core_ids=list(range(N_CORES)))
    outs = [
        np.asarray(res.results[c]["out"])
        .reshape(B_PER_CORE, H, W, N_SUMS)     # inverse of partition-major pack
        .astype(np.float32)
        for c in range(N_CORES)
    ]
    return np.concatenate(outs, axis=0)


# revision 18
# speedup vs baseline: 1.2207x; 1.2207x over previous
"""Trainium2 Bass kernel for nn_Conv2DSum (logconv1x1_2d / SPN sum layer).

Math: out[b,h,w,s] = logsumexp_c( x[b,h,w,c] + log_softmax(acc)[c,s] )
Since w = softmax(acc) along c sums to 1, the result equals
    out = log( exp(x) @ w )
which is a convex combination of exp(x_c) — numerically safe in fp32/fp16
range for N(0,1)-scale inputs (no max-subtraction needed).

V3 strategy (per core, batch-sharded 8 ways: 4 batches = 65536 rows x 32 ch),
memory-regime: all HBM I/O in fp16 (half the bytes of the fp32 baseline),
PE runs fp16 (1 cycle/row vs 4 for fp32), and the exp() is computed on the
otherwise-idle DVE via the classic float bit-trick so ACT only runs the
exact Ln:

  - x fp16 tiles [128, 2048]; PE transpose of [128,128] slices puts
    (4 rows x 32 ch) on partitions; fp16 transpose writes fp16 PSUM, so a
    whole tile's transpose fits 2 PSUM banks.
  - exp via DVE: i16 = round(x * 1024/ln2 + (15-sigma)*1024) written as
    int16; those bits reinterpreted as fp16 are ~exp(x) (max ~4% relative,
    deterministic, mostly averaged out by the 32-channel weighted sum).
    One tensor_scalar per tile, PSUM fp16 -> SBUF int16.
  - one matmul per [128,128] slice: stationary = p~ (bitcast fp16),
    moving = 128x128 block-diagonal weight (4 copies of the 32x32 softmax
    matrix) so 4 row-groups go in a single K=128,M=128,N=128 fp16 matmul.
  - exact Ln via ScalarE ACT (PSUM fp32 -> SBUF fp16), [128,1024] per
    2-bank PSUM group.
  - out fp16 -> HBM; host widens to fp32.

End-to-end rel err (host-simulated + device-verified) ~8.7e-3 vs the 2e-2
gate, dominated by the exp bit-trick; exact-exp fallback (USE_TRICK_EXP =
False) runs exp on ACT instead (~1.5e-3 but ~9us slower).
"""

from contextlib import ExitStack

import numpy as np

import concourse.bass as bass
import concourse.tile as tile
from concourse import mybir

# Problem shape (hardcoded per contest rules)
B, H, W, C_IN, N_SUMS = 32, 128, 128, 32, 32
N_CORES = 8
B_PER_CORE = B // N_CORES              # 4
ROWS_PER_CORE = B_PER_CORE * H * W     # 65536
FREE = 2048                            # big-tile free dim (64 rows x 32 ch)
N_TILES = ROWS_PER_CORE * C_IN // (128 * FREE)   # 8
N_SLICES = FREE // 128                 # 16 slices of [128,128] per big tile
SLICES_PER_BANK = 8                    # 8 slices per [128,1024] fp32 PSUM group
N_BANKGROUPS = N_SLICES // SLICES_PER_BANK       # 2

F32 = mybir.dt.float32
F16 = mybir.dt.float16
I16 = mybir.dt.int16

USE_TRICK_EXP = True

# exp(x) ~= bitcast_fp16(int16(round(A16*x + B16)))
_SIGMA = 0.0455
A16 = 1024.0 / float(np.log(2.0))
B16 = (15.0 - _SIGMA) * 1024.0


# Per-core HBM layout is partition-major: [128, 16384] fp16 where partition
# p owns rows [p*512, (p+1)*512) of the core's 65536 rows (each row = 32 ch).
# This gives 32KB contiguous HBM per partition, so a handful of big DMAs
# (4-8KB contiguous per partition each) replace 32 small ones: DMA_DIRECT2D
# costs ~600ns of Sync-engine time apiece, and 2KB-run descriptors kept the
# DMA engines at only ~57% busy in the V3 trace.
TOTAL_FREE = N_TILES * FREE            # 16384
# (col_offset, width) of each input DMA; first two small so compute starts
# after ~0.25MB instead of 1MB. All chunks stay resident in SBUF (32KB per
# partition total), so every input DMA issues with no WAR wait and the
# in-order Sync queue never head-of-line blocks an input load behind an
# output store.
X_CHUNKS = [(0, 2048), (2048, 2048), (4096, 4096), (8192, 8192)]
C_TILES = [(c, 2048) for c in range(0, 16384, 2048)]


def build_kernel(nc: bass.Bass, repeat: int = 1):
    x_d = nc.dram_tensor("x", [128, TOTAL_FREE], F16, kind="ExternalInput").ap()
    wblk_d = nc.dram_tensor("w_blk", [128, 128], F16, kind="ExternalInput").ap()
    ident_d = nc.dram_tensor("ident", [128, 128], F16, kind="ExternalInput").ap()
    out_d = nc.dram_tensor("out", [128, TOTAL_FREE], F16, kind="ExternalOutput").ap()

    with tile.TileContext(nc) as tc, ExitStack() as ctx:
        const_pool = ctx.enter_context(tc.tile_pool(name="const", bufs=1))
        # SBUF is ~64KB/partition usable here; pool slots are sized to the
        # largest tile, so segregate x chunks by width to stay within budget.
        x_pools = {}
        for width in sorted({w for _, w in X_CHUNKS}):
            n = sum(1 for _, w in X_CHUNKS if w == width)
            x_pools[width] = ctx.enter_context(
                tc.tile_pool(name=f"x{width}", bufs=n)
            )
        p_pool = ctx.enter_context(tc.tile_pool(name="p", bufs=2))
        o_pool = ctx.enter_context(tc.tile_pool(name="o", bufs=3))
        # psT: whole-tile fp16 transpose = 4KB = 2 banks; psO: [128,1024] fp32
        # = 2 banks. 2 bufs each -> all 8 banks.
        psT_pool = ctx.enter_context(tc.tile_pool(name="psT", bufs=2, space="PSUM"))
        psO_pool = ctx.enter_context(tc.tile_pool(name="psO", bufs=2, space="PSUM"))

        def load_chunk(c):
            off, width = X_CHUNKS[c]
            xt = x_pools[width].tile([128, width], F16, tag=f"x{c}")
            nc.sync.dma_start(xt[:], x_d[:, off : off + width])
            return xt

        # ident gates the first transpose and is tiny: load it first, then
        # x chunk 0, then wblk (not needed until the first weight MM), then
        # the remaining chunks — all unblocked, so the DMA engines saturate
        # on input immediately.
        ident = const_pool.tile([128, 128], F16, tag="ident")
        nc.sync.dma_start(ident[:], ident_d)
        xc0 = load_chunk(0)
        wblk = const_pool.tile([128, 128], F16, tag="wblk")
        nc.sync.dma_start(wblk[:], wblk_d)

        # walrus only allows ONE embedded sync-wait on a Matmult (the wait
        # rides the LDW struct), but Tile routinely needs 2+ on the first
        # matmul of a burst. A dummy bf16 ldweights is a PE *engine*
        # instruction (so its waits advance the PE engine's vector clock,
        # unlike a sequencer nop) with no PSUM side effects: we front-load
        # each matmul burst's cross-engine deps onto dummy LDWs, one dep
        # each, leaving at most the PSUM-bank WAW wait on the matmul itself.
        dummy_w = const_pool.tile([128, 8], mybir.dt.bfloat16, tag="dummyw")
        nc.gpsimd.memset(dummy_w[:], 1.0)

        # tiny dummy activation up front: forces the ~2.7us ACT table load
        # to overlap the first x DMA instead of sitting on the critical path
        warm_pool = ctx.enter_context(tc.tile_pool(name="warm", bufs=1))
        warm = warm_pool.tile([128, 1], F32, tag="warm")
        nc.scalar.activation(
            warm[:], dummy_w[:, 0:1], mybir.ActivationFunctionType.Ln
        )
        if not USE_TRICK_EXP:
            nc.scalar.activation(
                warm[:], dummy_w[:, 0:1], mybir.ActivationFunctionType.Exp
            )

        def chunk_of(col):
            for ci, (off, width) in enumerate(X_CHUNKS):
                if off <= col < off + width:
                    return ci, col - off
            raise AssertionError(col)

        for _rep in range(repeat):
            chunk_bufs = {0: xc0 if _rep == 0 else load_chunk(0)}
            for ci in range(1, len(X_CHUNKS)):
                chunk_bufs[ci] = load_chunk(ci)

            n_ct = len(C_TILES)

            def emit_transposes(t):
                col, width = C_TILES[t]
                psT = psT_pool.tile([128, width], F16)
                for k in range(width // 128):
                    ci, coff = chunk_of(col + k * 128)
                    nc.tensor.matmul(
                        psT[:, bass.ts(k, 128)],
                        chunk_bufs[ci][:, coff : coff + 128],
                        ident[:],
                        is_transpose=True,
                        start=(k % 4 == 0),
                        stop=(k % 4 == 3),
                    )
                return psT

            # software-pipelined PE stream: transposes run one tile ahead of
            # the weight matmuls so the in-order PE queue never stalls on the
            # DVE exp of the current tile.
            psTs = {0: emit_transposes(0), 1: emit_transposes(1)}
            for t in range(n_ct):
                col, width = C_TILES[t]
                psT = psTs.pop(t)
                if USE_TRICK_EXP:
                    pt = p_pool.tile([128, width], I16)
                    nc.vector.tensor_scalar(
                        pt[:],
                        psT[:],
                        A16,
                        B16,
                        op0=mybir.AluOpType.mult,
                        op1=mybir.AluOpType.add,
                    )
                    ptv = pt[:].bitcast(F16)
                else:
                    pt = p_pool.tile([128, width], F16)
                    nc.scalar.activation(
                        pt[:], psT[:], mybir.ActivationFunctionType.Exp
                    )
                    ptv = pt[:]
                ot = o_pool.tile([128, width], F16)
                for b in range(width // (128 * SLICES_PER_BANK)):
                    psO = psO_pool.tile([128, 128 * SLICES_PER_BANK], F32)
                    for k in range(SLICES_PER_BANK):
                        j = b * SLICES_PER_BANK + k
                        nc.tensor.matmul(
                            psO[:, bass.ts(k, 128)],
                            ptv[:, bass.ts(j, 128)],
                            wblk[:],
                            start=(k % 4 == 0),
                            stop=(k % 4 == 3),
                        )
                    nc.scalar.activation(
                        ot[:, bass.ts(b, 128 * SLICES_PER_BANK)],
                        psO[:],
                        mybir.ActivationFunctionType.Ln,
                    )
                nc.sync.dma_start(out_d[:, col : col + width], ot[:])
                # hoist tile t+2's transposes here (after MM2_t in the PE
                # stream): MM2_t isn't delayed, and the transposes fill the
                # PE's wait for DVE_{t+1} instead of idling.
                if t + 2 < n_ct:
                    psTs[t + 2] = emit_transposes(t + 2)
    return nc


# walrus rejects >1 embedded sync-wait on engine-instruction structs
# (Matmult/Activation/DMA...). The NX sequencer executes embedded waits in
# stream order anyway, so spilling all-but-one wait onto dedicated nops
# immediately before the instruction is semantically identical.
_SPLIT_TYPES = (
    "InstMatmult",
    "InstLdweights",
    "InstActivation",
    "InstDMACopy",
    "InstMemset",
    "InstTensorTensor",
    "InstTensorScalarPtr",
    "InstCopy",
    "InstTensorReduce",
    "InstDrain",
    "InstNoOp",
)


def _split_embedded_waits(nc: bass.Bass):
    for fn in nc.m.functions:
        for blk in fn.blocks:
            insts = blk.instructions
            out = []
            for inst in insts:
                si = inst.sync_info
                if (
                    si is not None
                    and si.on_wait
                    and len(si.on_wait) > 1
                    and type(inst).__name__ in _SPLIT_TYPES
                ):
                    waits = list(si.on_wait)
                    for i, w in enumerate(waits[:-1]):
                        nop = mybir.InstNoOp(
                            name=f"{inst.name}-sw{i}",
                            engine=inst.engine,
                            sync_info=mybir.SyncInfo(on_wait=[w], on_update=[]),
                            bass_nofuse=True,
                        )
                        out.append(nop)
                    inst.sync_info = mybir.SyncInfo(
                        on_wait=[waits[-1]], on_update=list(si.on_update)
                    )
                out.append(inst)
            if len(out) != len(insts):
                blk.instructions[:] = out


def _host_weights(accumulators: np.ndarray) -> np.ndarray:
    """log_softmax over c of [1,1,Cin,S] accumulators -> exp -> block-diag."""
    acc = np.asarray(accumulators, dtype=np.float64)[0, 0]      # [Cin, S]
    m = acc.max(axis=0, keepdims=True)
    e = np.exp(acc - m)
    w = (e / e.sum(axis=0, keepdims=True)).astype(np.float16)   # [Cin, S]
    w_blk = np.zeros((128, 128), dtype=np.float16)
    for g in range(4):
        w_blk[32 * g : 32 * g + 32, 32 * g : 32 * g + 32] = w
    return w_blk


def make_in_maps(x: np.ndarray, acc: np.ndarray) -> list[dict]:
    x16 = np.ascontiguousarray(np.asarray(x).astype(np.float16))
    w_blk = _host_weights(np.asarray(acc, dtype=np.float32))
    ident = np.eye(128, dtype=np.float16)
    in_maps = []
    for c in range(N_CORES):
        xs = x16[c * B_PER_CORE : (c + 1) * B_PER_CORE]     # [4,128,128,32]
        # partition-major: partition p owns rows [p*512, (p+1)*512)
        xs = xs.reshape(128, TOTAL_FREE)
        in_maps.append({"x": xs, "w_blk": w_blk, "ident": ident})
    return in_maps


_CACHE: dict = {}


def make_bass():
    return bass.Bass("TRN2", debug=False, num_swdge_queues=4)


def get_nc():
    if "nc" not in _CACHE:
        nc = build_kernel(make_bass())
        # HW path only: CoreSim can't digest post-hoc inserted nops
        _split_embedded_waits(nc)
        _CACHE["nc"] = nc
    return _CACHE["nc"]


def kernel(**inputs: np.ndarray) -> np.ndarray:
    from concourse.bass_utils import run_bass_kernel_spmd

    in_maps = make_in_maps(inputs["x"], inputs["accumulators"])
    nc = get_nc()
    res = run_bass_kernel_spmd(nc, in_maps, core_ids=list(range(N_CORES)))
    outs = [
        np.asarray(res.results[c]["out"])
        .reshape(B_PER_CORE, H, W, N_SUMS)     # inverse of partition-major pack
        .astype(np.float32)
        for c in range(N_CORES)
    ]
    return np.concatenate(outs, axis=0)
